# revision 99
# baseline (speedup 1.0000x reference)
"""MoE routing mixture kernel for Trainium2 (8 NeuronCores, SPMD data-parallel).

Math: out[b] = sum_k selection_score[b, idx[b,k]] * all_weight[idx[b,k]]
Rewritten as a dense matmul: out = C @ W_flat, where
  C[b,e]    = selection_score[b,e] * |{k : idx[b,k]==e}|      ([2048, 64])
  W_flat    = all_weight.reshape(64, 16384)
Sharding: batch rows split across 8 cores (256 rows each); W replicated.

Raw Bass (no Tile): this toolchain's descriptors carry at most one sync wait
and one sync update each, so all synchronization is standalone wait_ge
instructions plus .then_inc updates, one per instruction.

Pipeline per core:
  SP   : 6 small input DMAs -> 4 W-chunk DMAs -> 16 output DMAs (1 MiB each)
  DVE  : C = score * count(idx==e) per 128-row chunk; C^T copies from PSUM
  PE   : 2 transposes (C -> C^T), then 64 matmuls [64x128]@[64x512] -> PSUM
  ACT  : 64 PSUM->SBUF copies into 16 staging tiles (no slot reuse)
"""

import sys
from contextlib import ExitStack

import numpy as np

sys.path.insert(0, "/opt/trn_rl_repo")

BS, E, TOPK, PL, D = 2048, 64, 8, 32, 512
NF = PL * D  # 16384 flattened prompt*dim
N_CORES = 8
RPC = BS // N_CORES  # 256 rows per core
RCHUNKS = RPC // 128  # 2 row chunks of 128
HALF = NF // 2  # 8192: W stored on-chip as [128, 8192]
WCHUNKS = 8  # W loaded in 8 chunks of [128, 1024]
WCW = HALF // WCHUNKS  # 2048
SLICES = WCW // D  # 4 matmuls (512 cols) per (chunk, half)
NPSUM = 6  # matmul PSUM ring
NGRP = WCHUNKS * RCHUNKS * 2  # 16 staging groups of [128, 2048]

_cache: dict = {}


def _build_program():
    import concourse.bass as bass
    import concourse.mybir as mybir

    f32 = mybir.dt.float32
    nc = bass.Bass()

    scores_d = nc.declare_dram_parameter("scores", [RPC, E], f32, isOutput=False)
    idx_d = nc.declare_dram_parameter("idxf", [RPC, TOPK], f32, isOutput=False)
    # W_flat [64, 16384] host-rearranged to [128, 8192]:
    # partition h*64+e holds cols [h*8192, (h+1)*8192) of expert e.
    wk_d = nc.declare_dram_parameter("wk", [128, HALF], f32, isOutput=False)
    iota_d = nc.declare_dram_parameter("iota", [128, E], f32, isOutput=False)
    ident_d = nc.declare_dram_parameter("ident", [128, 128], f32, isOutput=False)
    out_d = nc.declare_dram_parameter("out", [RPC, NF], f32, isOutput=True)

    ctx = ExitStack()
    with ctx:
        f32r = mybir.dt.float32r
        sb = lambda shape, tag, dt=f32: ctx.enter_context(  # noqa: E731
            nc.sbuf_tensor(tag, shape, dt)
        )
        w_t = sb([128, HALF], "w_t")
        iota_t = sb([128, E], "iota_t")
        ident_t = sb([128, 128], "ident_t")
        sc_t = [sb([128, E], f"sc{r}") for r in range(RCHUNKS)]
        idx_t = [sb([128, TOPK], f"idx{r}") for r in range(RCHUNKS)]
        eqs = [sb([128, E], f"eq{i}") for i in range(TOPK)]
        prs = [sb([128, E], f"pr{i}") for i in range(TOPK // 2)]
        qds = [sb([128, E], f"qd{i}") for i in range(TOPK // 4)]
        cnt = [sb([128, E], f"cnt{r}") for r in range(RCHUNKS)]
        ct = [sb([128, 128], f"ct{r}") for r in range(RCHUNKS)]
        stg = [sb([128, WCW], f"stg{g}") for g in range(NGRP)]

        ctp = [
            ctx.enter_context(nc.psum_tensor(f"ctp{r}", [E, 128], f32))
            for r in range(RCHUNKS)
        ]
        pmm = [
            ctx.enter_context(nc.psum_tensor(f"pmm{i}", [128, D], f32))
            for i in range(NPSUM)
        ]

        s_in = ctx.enter_context(nc.semaphore("s_in"))
        s_w = ctx.enter_context(nc.semaphore("s_w"))
        s_dve = ctx.enter_context(nc.semaphore("s_dve"))
        s_pe = ctx.enter_context(nc.semaphore("s_pe"))
        s_act = ctx.enter_context(nc.semaphore("s_act"))
        s_cpv = ctx.enter_context(nc.semaphore("s_cpv"))
        s_out = ctx.enter_context(nc.semaphore("s_out"))

        # matmul m (PE order) -> (wchunk c, rowchunk rc, half h, slice s)
        def mm_seq():
            m = 0
            for c in range(WCHUNKS):
                for rc in range(RCHUNKS):
                    for h in range(2):
                        for s in range(SLICES):
                            yield m, c, rc, h, s
                            m += 1

        N_MM = WCHUNKS * RCHUNKS * 2 * SLICES  # 64

        block = ctx.enter_context(nc.Block())

        @block.sync
        def _(sp):
            sp.dma_start(out=iota_t[:], in_=iota_d[:]).then_inc(s_in, 16)
            sp.dma_start(out=ident_t[:], in_=ident_d[:]).then_inc(s_in, 16)
            for r in range(RCHUNKS):
                rows = slice(r * 128, (r + 1) * 128)
                sp.dma_start(out=sc_t[r][:], in_=scores_d[rows, :]).then_inc(s_in, 16)
                sp.dma_start(out=idx_t[r][:], in_=idx_d[rows, :]).then_inc(s_in, 16)
            for c in range(WCHUNKS):
                cols = slice(c * WCW, (c + 1) * WCW)
                sp.dma_start(out=w_t[:, cols], in_=wk_d[:, cols]).then_inc(s_w, 16)

        @block.vector
        def _(v):
            v.wait_ge(s_in, 96)
            for r in range(RCHUNKS):
                for k in range(TOPK):
                    v.tensor_scalar(
                        eqs[k][:],
                        iota_t[:],
                        idx_t[r][:, k : k + 1],
                        None,
                        mybir.AluOpType.is_equal,
                    )
                v.drain()
                for i in range(TOPK // 2):
                    v.tensor_add(prs[i][:], eqs[2 * i][:], eqs[2 * i + 1][:])
                v.drain()
                for i in range(TOPK // 4):
                    v.tensor_add(qds[i][:], prs[2 * i][:], prs[2 * i + 1][:])
                v.drain()
                v.tensor_add(cnt[r][:], qds[0][:], qds[1][:])
                v.drain()
                v.tensor_mul(cnt[r][:], cnt[r][:], sc_t[r][:]).then_inc(s_dve, 1)
            for r in range(RCHUNKS):
                v.wait_ge(s_pe, r + 1)
                v.tensor_copy(ct[r][:E, :], ctp[r][:]).then_inc(s_dve, 1)
                v.tensor_copy(ct[r][E:, :], ctp[r][:]).then_inc(s_dve, 1)
            # odd-m PSUM->SBUF copies (evens go to ACT)
            for m, c, rc, h, s in mm_seq():
                if m % 2 == 0:
                    continue
                v.wait_ge(s_pe, RCHUNKS + m + 1)
                gi = c * (RCHUNKS * 2) + rc * 2 + h
                v.tensor_copy(
                    stg[gi][:, s * D : (s + 1) * D], pmm[m % NPSUM][:]
                ).then_inc(s_cpv, 1)

        @block.tensor
        def _(t):
            t.wait_ge(s_in, 96)  # ident
            for r in range(RCHUNKS):
                t.wait_ge(s_dve, r + 1)
                t.transpose(ctp[r][:], cnt[r][:], ident_t[:]).then_inc(s_pe, 1)
            t.wait_ge(s_dve, RCHUNKS + 2 * RCHUNKS)  # all ct copies done
            cur_c = -1
            for m, c, rc, h, s in mm_seq():
                if c != cur_c:
                    t.wait_ge(s_w, 16 * (c + 1))
                    cur_c = c
                if m >= NPSUM:
                    mm = m - NPSUM
                    if mm % 2 == 0:
                        t.wait_ge(s_act, mm // 2 + 1)
                    else:
                        t.wait_ge(s_cpv, mm // 2 + 1)
                pslice = slice(h * E, (h + 1) * E)
                wc = c * WCW + s * D
                t.matmul(
                    pmm[m % NPSUM][:],
                    ct[rc][pslice, :],
                    w_t[pslice, wc : wc + D],
                    start=True,
                    stop=True,
                ).then_inc(s_pe, 1)

        @block.scalar
        def _(a):
            for m, c, rc, h, s in mm_seq():
                if m % 2 == 1:
                    continue
                a.wait_ge(s_pe, RCHUNKS + m + 1)
                gi = c * (RCHUNKS * 2) + rc * 2 + h
                a.copy(
                    stg[gi][:, s * D : (s + 1) * D], pmm[m % NPSUM][:]
                ).then_inc(s_act, 1)

        @block.gpsimd
        def _(gp):
            # Output stores on SWDGE: group gi ready when its 2 ACT + 2 DVE
            # copies are done.
            gi = 0
            for c in range(WCHUNKS):
                for rc in range(RCHUNKS):
                    for h in range(2):
                        rows = slice(rc * 128, (rc + 1) * 128)
                        colbase = h * HALF + c * WCW
                        gp.wait_ge(s_act, (SLICES // 2) * (gi + 1))
                        gp.wait_ge(s_cpv, (SLICES // 2) * (gi + 1))
                        gp.dma_start(
                            out=out_d[rows, colbase : colbase + WCW],
                            in_=stg[gi][:],
                        ).then_inc(s_out, 16)
                        gi += 1
            gp.wait_ge(s_out, 16 * NGRP)

    return nc


def _run(selection_score, expert_indices, all_weight, trace=False):
    from concourse.bass_utils import run_bass_kernel_spmd

    scores = np.ascontiguousarray(np.asarray(selection_score, dtype=np.float32))
    idxf = np.ascontiguousarray(np.asarray(expert_indices).astype(np.float32))
    w = np.asarray(all_weight, dtype=np.float32).reshape(E, NF)
    wk = np.ascontiguousarray(
        w.reshape(E, 2, HALF).transpose(1, 0, 2).reshape(128, HALF)
    )
    iota = np.ascontiguousarray(np.tile(np.arange(E, dtype=np.float32), (128, 1)))
    ident = np.eye(128, dtype=np.float32)

    if "nc" not in _cache:
        _cache["nc"] = _build_program()
    nc = _cache["nc"]

    in_maps = [
        {
            "scores": np.ascontiguousarray(scores[c * RPC : (c + 1) * RPC]),
            "idxf": np.ascontiguousarray(idxf[c * RPC : (c + 1) * RPC]),
            "wk": wk,
            "iota": iota,
            "ident": ident,
        }
        for c in range(N_CORES)
    ]
    r = run_bass_kernel_spmd(nc, in_maps, list(range(N_CORES)), trace=trace)
    full = np.concatenate([r.results[c]["out"] for c in range(N_CORES)], axis=0)
    return full.reshape(BS, PL, D).astype(np.float32, copy=False), r


def kernel(selection_score, expert_indices, all_weight) -> np.ndarray:
    full, _ = _run(selection_score, expert_indices, all_weight, trace=False)
    return full



# revision 100
# speedup vs baseline: 1.3602x; 1.3602x over previous
"""MoE routing mixture kernel for Trainium2 — feature-sharded bf16 design.

out = C @ W_flat with C[b,e] = score[b,e] * count(idx[b,:]==e).
Core c computes all 2048 rows x its 2048-col slice. Stores are the DMA
roofline (8 MiB bf16/core). Host upcasts bf16 output to f32.

Engine roles (all HW-legal: GPSIMD never touches PSUM):
  SP  : input DMAs + output DMAs (HWDGE)
  DVE : broadcast is_equal (one-hot planes), C=count*score muls,
        part of PSUM->SBUF staging copies
  POOL: k-sum add tree over the one-hot planes (SBUF only)
  PE  : 8 plain pair-transposes C -> C^T (bf16), 64 bf16 matmuls
  ACT : C^T PSUM->SBUF copies + whole-bank staging copies
"""

import sys
from contextlib import ExitStack

import numpy as np

sys.path.insert(0, "/opt/trn_rl_repo")

BS, E, TOPK, PL, D = 2048, 64, 8, 32, 512
NF = PL * D
N_CORES = 8
COLS = NF // N_CORES  # 2048
NCH = BS // 128  # 16
NT = NCH * 4  # 64 matmul tiles [128,512]
NSLOT = 6  # psum ring: 3 x [128,1024] f32
NPAIR = NCH // 2

EGROUPS = [[0, 1], [2, 3], [4, 5, 6, 7], [8, 9, 10, 11], [12, 13, 14, 15]]
EGROUP_OF = {c: g for g, chs in enumerate(EGROUPS) for c in chs}
IDX_IN_G = {c: i for chs in EGROUPS for i, c in enumerate(chs)}

# misc input layout (bf16): [idx 128 | iota_x 512 | ident 128] = 768 cols
MISC_COLS = 768
DVE_CHUNKS = tuple(range(3, 14))  # their (q2,q3) tiles copied by DVE
DVE_TILES = tuple(4 * c + q for c in DVE_CHUNKS for q in (2, 3))

_cache: dict = {}


def _build_program():
    import concourse.bass as bass
    import concourse.mybir as mybir
    from concourse.ap import AP

    bf16 = mybir.dt.bfloat16
    f32 = mybir.dt.float32
    nc = bass.Bass()

    misc_d = nc.declare_dram_parameter("misc", [128, MISC_COLS], bf16, isOutput=False)
    sc_d = nc.declare_dram_parameter("sc", [128, NCH * E], bf16, isOutput=False)
    wk_d = nc.declare_dram_parameter("wk", [128, COLS], bf16, isOutput=False)
    out_d = nc.declare_dram_parameter("out", [BS, COLS], bf16, isOutput=True)

    ctx = ExitStack()
    with ctx:
        sb = lambda tag, shape, dt=bf16: ctx.enter_context(  # noqa: E731
            nc.sbuf_tensor(tag, shape, dt)
        )
        misc_t = sb("misc_t", [128, MISC_COLS])
        idx_t = misc_t[:, 0:128]
        iota_x = misc_t[:, 128:640]
        ident = misc_t[:, 640:768]
        sc_t = sb("sc_t", [128, NCH * E])
        w_t = sb("w_t", [128, COLS])
        eqb = [sb(f"eq{i}", [128, 2048]) for i in range(2)]
        t4 = sb("t4", [128, 1024])
        t2 = sb("t2", [128, 512])
        cnt = sb("cnt", [128, NCH * E])
        cbuf = sb("cbuf", [128, NCH * E])
        ct = sb("ct", [128, NPAIR * 128])
        stg = [sb(f"stg{c}", [128, COLS]) for c in range(NCH)]

        pct = [
            ctx.enter_context(nc.psum_tensor(f"pct{i}", [128, 512], bf16))
            for i in range(2)
        ]
        pmm = [
            ctx.enter_context(nc.psum_tensor(f"pmm{i}", [128, 1024], f32))
            for i in range(3)
        ]

        s_ld = [ctx.enter_context(nc.semaphore(f"s_ld{i}")) for i in range(4)]
        s_dve = ctx.enter_context(nc.semaphore("s_dve"))
        s_pool = ctx.enter_context(nc.semaphore("s_pool"))
        s_pe = ctx.enter_context(nc.semaphore("s_pe"))
        s_cpa = ctx.enter_context(nc.semaphore("s_cpa"))
        s_cpv = ctx.enter_context(nc.semaphore("s_cpv"))
        s_out = ctx.enter_context(nc.semaphore("s_out"))

        def pstr(t):
            return t[:].ap[0][0]

        # ---- positions -----------------------------------------------
        # s_dve: per group: eq=+1; then per pair of group: mul C=+1
        dve_pos = {}
        dp = 0
        for g, chs in enumerate(EGROUPS):
            dp += 1
            dve_pos[("eq", g)] = dp
            for pr in sorted({c // 2 for c in chs}):
                dp += 1
                dve_pos[("C", pr)] = dp
        # s_pool: one inc per group (a3 done)
        pool_pos = {("cnt", g): g + 1 for g in range(len(EGROUPS))}
        # s_pe: per pair tr + per tile mm, group-blocked order
        pe_pos = {}
        pe_order = []
        p = 0
        for chs in EGROUPS:
            for pr in sorted({c // 2 for c in chs}):
                p += 1
                pe_pos[("tr", pr)] = p
                pe_order.append(("tr", pr))
            for c in chs:
                for q in range(4):
                    p += 1
                    pe_pos[("mm", 4 * c + q)] = p
                    pe_order.append(("mm", c, q))

        # copy plan: ACT dbls per bank except DVE chunks' h1 pair
        act_ops = []  # ("ctcp", pr) | ("dbl", c, h)
        ctcp_done = set()
        for c in range(NCH):
            pr = c // 2
            if pr not in ctcp_done:
                ctcp_done.add(pr)
                act_ops.append(("ctcp", pr))
            for h in range(2):
                if c in DVE_CHUNKS and h == 1:
                    continue  # DVE singles
                act_ops.append(("dbl", c, h))
        act_pos = {op: i + 1 for i, op in enumerate(act_ops)}

        tile_event = {}
        for op in act_ops:
            if op[0] == "dbl":
                _, c, h = op
                for t in (4 * c + 2 * h, 4 * c + 2 * h + 1):
                    tile_event[t] = ("scalar", act_pos[op])
        vcum = 0
        vpos = {}
        for t in DVE_TILES:
            vcum += 1
            vpos[t] = vcum
            tile_event[t] = ("vector", vcum)
        SEMS = {"scalar": s_cpa, "vector": s_cpv}

        # DVE copy weave: tiles of chunk c after mul C(pair(c+2)); both
        # q2/q3 singles. Safe: a group's trs precede its mms in PE order.
        weave = {}
        wtail = []
        for t in DVE_TILES:
            c_t = t // 4
            if t + NSLOT >= NT:
                wtail.append(t)
            else:
                weave.setdefault(min((c_t + 2) // 2, NPAIR - 1), []).append(t)

        def pct_tile(pr):
            return pct[pr % 2][:, (pr // 2) * 128 : (pr // 2) * 128 + 128]

        def pmm_slice(t, n=1):
            slot = t % NSLOT
            return pmm[slot // 2][:, (slot % 2) * 512 : (slot % 2) * 512 + 512 * n]

        def stg_slice(t, n=1):
            c, q = t // 4, t % 4
            return stg[c][:, q * 512 : (q + n) * 512]

        block = ctx.enter_context(nc.Block())

        @block.sync
        def _(sp):
            sp.dma_start(out=misc_t[:], in_=misc_d[:]).then_inc(s_ld[0], 16)
            sp.dma_start(out=sc_t[:], in_=sc_d[:]).then_inc(s_ld[1], 16)
            sp.dma_start(out=w_t[:, :1024], in_=wk_d[:, :1024]).then_inc(s_ld[2], 16)
            sp.dma_start(out=w_t[:, 1024:], in_=wk_d[:, 1024:]).then_inc(s_ld[3], 16)
            n_stores = 0
            for c in range(NCH):
                rows = slice(c * 128, (c + 1) * 128)
                pieces = [(0, 1024), (1024, 1024)] if c < 2 else [(0, COLS)]
                for col0, w in pieces:
                    needs: dict = {}
                    for t in range(col0 // 512, (col0 + w) // 512):
                        sem_name, thr = tile_event[4 * c + t]
                        needs[sem_name] = max(needs.get(sem_name, 0), thr)
                    for sem_name, thr in needs.items():
                        sp.wait_ge(SEMS[sem_name], thr)
                    sp.dma_start(
                        out=out_d[rows, col0 : col0 + w],
                        in_=stg[c][:, col0 : col0 + w],
                    ).then_inc(s_out, 16)
                    n_stores += 1
            sp.wait_ge(s_out, 16 * n_stores)

        @block.vector
        def _(v):
            def vcopy(t):
                v.wait_ge(s_pe, pe_pos[("mm", t)])
                v.tensor_copy(stg_slice(t), pmm_slice(t)).then_inc(s_cpv, 1)

            first = True
            seen_sc = False
            for g, chs in enumerate(EGROUPS):
                n = len(chs)
                ch0 = chs[0]
                if first:
                    v.wait_ge(s_ld[0], 16)
                    first = False
                if g >= 2:  # eq buffer ring-2 reuse: Pool done with g-2
                    v.wait_ge(s_pool, pool_pos[("cnt", g - 2)])
                eb = eqb[g % 2]
                in0 = AP(misc_t[:].tensor, ch0 * TOPK,
                         [[pstr(misc_t), 128], [TOPK, n], [0, E], [1, TOPK]])
                in1 = AP(misc_t[:].tensor, 128,
                         [[pstr(misc_t), 128], [0, n], [1, E * TOPK]])
                out_eq = AP(eb[:].tensor, 0,
                            [[pstr(eb), 128], [E * TOPK, n], [1, E * TOPK]])
                v.tensor_tensor(
                    out_eq, in0, in1, op=mybir.AluOpType.is_equal
                ).then_inc(s_dve, 1)
                # C = cnt * sc once Pool's tree lands
                if not seen_sc:
                    v.wait_ge(s_ld[1], 16)
                    seen_sc = True
                v.wait_ge(s_pool, pool_pos[("cnt", g)])
                cols = slice(ch0 * E, (chs[-1] + 1) * E)
                v.tensor_mul(cbuf[:, cols], cnt[:, cols], sc_t[:, cols])
                v.drain()
                for pr in sorted({c // 2 for c in chs}):
                    v.sem_inc(s_dve, 1)
                    for t in weave.get(pr, ()):
                        vcopy(t)
            for t in wtail:
                vcopy(t)

        @block.gpsimd
        def _(gp):
            for g, chs in enumerate(EGROUPS):
                n = len(chs)
                ch0 = chs[0]
                gp.wait_ge(s_dve, dve_pos[("eq", g)])
                eb = eqb[g % 2]
                a_in0 = AP(eb[:].tensor, 0,
                           [[pstr(eb), 128], [E * TOPK, n], [TOPK, E], [2, 4]])
                a_in1 = AP(eb[:].tensor, 1,
                           [[pstr(eb), 128], [E * TOPK, n], [TOPK, E], [2, 4]])
                a_out = AP(t4[:].tensor, 0,
                           [[pstr(t4), 128], [E * 4, n], [1, E * 4]])
                gp.tensor_tensor(a_out, a_in0, a_in1, op=mybir.AluOpType.add)
                gp.drain()
                b_in0 = AP(t4[:].tensor, 0,
                           [[pstr(t4), 128], [E * 4, n], [4, E], [2, 2]])
                b_in1 = AP(t4[:].tensor, 1,
                           [[pstr(t4), 128], [E * 4, n], [4, E], [2, 2]])
                b_out = AP(t2[:].tensor, 0,
                           [[pstr(t2), 128], [E * 2, n], [1, E * 2]])
                gp.tensor_tensor(b_out, b_in0, b_in1, op=mybir.AluOpType.add)
                gp.drain()
                c_in0 = AP(t2[:].tensor, 0,
                           [[pstr(t2), 128], [E * 2, n], [2, E]])
                c_in1 = AP(t2[:].tensor, 1,
                           [[pstr(t2), 128], [E * 2, n], [2, E]])
                c_out = AP(cnt[:].tensor, ch0 * E,
                           [[pstr(cnt), 128], [E, n], [1, E]])
                gp.tensor_tensor(
                    c_out, c_in0, c_in1, op=mybir.AluOpType.add
                ).then_inc(s_pool, 1)
                gp.drain()

        @block.tensor
        def _(t):
            for item in pe_order:
                if item[0] == "tr":
                    pr = item[1]
                    t.wait_ge(s_dve, dve_pos[("C", pr)])
                    if pr >= 2:  # pct ping-pong guard (see kernel notes)
                        t.wait_ge(s_cpa, act_pos[("ctcp", pr - 2)])
                    t.matmul(
                        pct_tile(pr),
                        cbuf[:, pr * 128 : (pr + 1) * 128],
                        ident,
                        is_transpose=True,
                        start=True,
                        stop=True,
                    ).then_inc(s_pe, 1)
                else:
                    _, c, q = item
                    tile = 4 * c + q
                    pr = c // 2
                    h = c % 2
                    if q == 0:
                        t.wait_ge(s_cpa, act_pos[("ctcp", pr)])
                    if tile < 2:
                        t.wait_ge(s_ld[2], 16)
                    if tile == 2:
                        t.wait_ge(s_ld[3], 16)
                    if tile >= NSLOT:
                        sem_name, thr = tile_event[tile - NSLOT]
                        t.wait_ge(SEMS[sem_name], thr)
                    t.matmul(
                        pmm_slice(tile),
                        ct[64 * h : 64 * h + 64, pr * 128 : (pr + 1) * 128],
                        w_t[64 * h : 64 * h + 64, q * 512 : (q + 1) * 512],
                        start=True,
                        stop=True,
                    ).then_inc(s_pe, 1)

        @block.scalar
        def _(a):
            for op in act_ops:
                if op[0] == "ctcp":
                    pr = op[1]
                    a.wait_ge(s_pe, pe_pos[("tr", pr)])
                    a.copy(
                        ct[:, pr * 128 : (pr + 1) * 128], pct_tile(pr)
                    ).then_inc(s_cpa, 1)
                else:
                    _, c, h = op
                    t0 = 4 * c + 2 * h
                    a.wait_ge(s_pe, pe_pos[("mm", t0 + 1)])
                    a.copy(stg_slice(t0, 2), pmm_slice(t0, 2)).then_inc(s_cpa, 1)

    return nc


def _host_prep(selection_score, expert_indices, all_weight):
    import ml_dtypes

    bf = ml_dtypes.bfloat16
    scores = np.asarray(selection_score, dtype=np.float32)
    idx = np.asarray(expert_indices).astype(np.int64)
    w = np.asarray(all_weight, dtype=np.float32).reshape(E, NF)

    idx_all = (
        idx.reshape(NCH, 128, TOPK).transpose(1, 0, 2).reshape(128, NCH * TOPK)
    ).astype(np.float32)
    iota = np.tile(np.repeat(np.arange(E, dtype=np.float32), TOPK), (128, 1))
    ident = np.eye(128, dtype=np.float32)
    misc = np.ascontiguousarray(
        np.concatenate([idx_all, iota, ident], axis=1).astype(bf)
    )
    sc = np.ascontiguousarray(
        scores.reshape(NCH, 128, E).transpose(1, 0, 2).reshape(128, NCH * E)
    ).astype(bf)
    wk_cores = []
    for core in range(N_CORES):
        sl = w[:, core * COLS : (core + 1) * COLS].astype(bf)
        wk_cores.append(np.ascontiguousarray(np.concatenate([sl, sl], axis=0)))
    return misc, sc, wk_cores


def _run(selection_score, expert_indices, all_weight, trace=False):
    from concourse.bass_utils import run_bass_kernel_spmd

    misc, sc, wk_cores = _host_prep(selection_score, expert_indices, all_weight)
    if "nc" not in _cache:
        _cache["nc"] = _build_program()
    nc = _cache["nc"]
    in_maps = [
        {"misc": misc, "sc": sc, "wk": wk_cores[c]} for c in range(N_CORES)
    ]
    r = run_bass_kernel_spmd(nc, in_maps, list(range(N_CORES)), trace=trace)
    full = np.concatenate(
        [np.asarray(r.results[c]["out"]) for c in range(N_CORES)], axis=1
    )
    return full.astype(np.float32).reshape(BS, PL, D), r


def kernel(selection_score, expert_indices, all_weight) -> np.ndarray:
    full, _ = _run(selection_score, expert_indices, all_weight, trace=False)
    return full


# revision 102
# speedup vs baseline: 1.6164x; 1.1883x over previous
"""MoE routing mixture kernel for Trainium2 — feature-sharded bf16 design.

out = C @ W_flat with C[b,e] = score[b,e] * count(idx[b,:]==e).
Core c computes all 2048 rows x its 2048-col slice. Stores are the DMA
roofline (8 MiB bf16/core). Host upcasts bf16 output to f32.

Engine roles (all HW-legal: GPSIMD never touches PSUM):
  SP  : input DMAs + output DMAs (HWDGE)
  DVE : broadcast is_equal (one-hot planes), C=count*score muls,
        part of PSUM->SBUF staging copies
  POOL: k-sum add tree over the one-hot planes (SBUF only)
  PE  : 8 plain pair-transposes C -> C^T (bf16), 64 bf16 matmuls
  ACT : C^T PSUM->SBUF copies + whole-bank staging copies
"""

import sys
from contextlib import ExitStack

import numpy as np

sys.path.insert(0, "/opt/trn_rl_repo")

BS, E, TOPK, PL, D = 2048, 64, 8, 32, 512
NF = PL * D
N_CORES = 8
COLS = NF // N_CORES  # 2048
NCH = BS // 128  # 16
NT = NCH * 4  # 64 matmul tiles [128,512]
NSLOT = 6  # psum ring: 3 x [128,1024] f32
NPAIR = NCH // 2

EGROUPS = [[2 * p, 2 * p + 1] for p in range(8)]
EGROUP_OF = {c: g for g, chs in enumerate(EGROUPS) for c in chs}
IDX_IN_G = {c: i for chs in EGROUPS for i, c in enumerate(chs)}

# misc input layout (bf16): [idx 128 | iota_x 512 | ident 128] = 768 cols
MISC_COLS = 768
DVE_CHUNKS = tuple(range(3, 14))  # their (q2,q3) tiles copied by DVE
DVE_TILES = tuple(4 * c + q for c in DVE_CHUNKS for q in (2, 3))

_cache: dict = {}


def _build_program():
    import concourse.bass as bass
    import concourse.mybir as mybir
    from concourse.ap import AP

    bf16 = mybir.dt.bfloat16
    f32 = mybir.dt.float32
    nc = bass.Bass()

    misc_d = nc.declare_dram_parameter("misc", [128, MISC_COLS], bf16, isOutput=False)
    sc_d = nc.declare_dram_parameter("sc", [128, NCH * E], bf16, isOutput=False)
    wk_d = nc.declare_dram_parameter("wk", [128, COLS], bf16, isOutput=False)
    out_d = nc.declare_dram_parameter("out", [BS, COLS], bf16, isOutput=True)

    ctx = ExitStack()
    with ctx:
        sb = lambda tag, shape, dt=bf16: ctx.enter_context(  # noqa: E731
            nc.sbuf_tensor(tag, shape, dt)
        )
        misc_t = sb("misc_t", [128, MISC_COLS])
        idx_t = misc_t[:, 0:128]
        iota_x = misc_t[:, 128:640]
        ident = misc_t[:, 640:768]
        sc_t = sb("sc_t", [128, NCH * E])
        w_t = sb("w_t", [128, COLS])
        eqb = [sb(f"eq{i}", [128, 2048]) for i in range(3)]
        t4 = [sb(f"t4_{i}", [128, 1024]) for i in range(2)]
        t2 = sb("t2", [128, 512])
        cnt = sb("cnt", [128, NCH * E])
        cbuf = sb("cbuf", [128, NCH * E])
        ct = sb("ct", [128, NPAIR * 128])
        stg = [sb(f"stg{c}", [128, COLS]) for c in range(NCH)]

        pct = [
            ctx.enter_context(nc.psum_tensor(f"pct{i}", [128, 512], bf16))
            for i in range(2)
        ]
        pmm = [
            ctx.enter_context(nc.psum_tensor(f"pmm{i}", [128, 1024], f32))
            for i in range(3)
        ]

        s_ld = [ctx.enter_context(nc.semaphore(f"s_ld{i}")) for i in range(4)]
        s_dve = ctx.enter_context(nc.semaphore("s_dve"))
        s_pool = ctx.enter_context(nc.semaphore("s_pool"))
        s_pe = ctx.enter_context(nc.semaphore("s_pe"))
        s_cpa = ctx.enter_context(nc.semaphore("s_cpa"))
        s_cpv = ctx.enter_context(nc.semaphore("s_cpv"))
        s_out = ctx.enter_context(nc.semaphore("s_out"))

        def pstr(t):
            return t[:].ap[0][0]

        # ---- positions -----------------------------------------------
        # s_dve emission order: eq+a1 for groups 0..2 up front (ring-3 eq
        # buffers), then per group: C incs per pair, with eq+a1 of group
        # g+3 woven in -- so Pool's tree(g) overlaps DVE's eq(g+1..3).
        dve_order = [("eqa1", 0), ("eqa1", 1)]
        for g, chs in enumerate(EGROUPS):
            for pr in sorted({c // 2 for c in chs}):
                dve_order.append(("C", pr))
            if g + 2 < len(EGROUPS):
                dve_order.append(("eqa1", g + 2))
        dve_pos = {op: i + 1 for i, op in enumerate(dve_order)}
        # s_pool: one inc per group (a3 done)
        pool_pos = {("cnt", g): g + 1 for g in range(len(EGROUPS))}

        def emit_eqa1(v, g, mybir, AP):
            chs = EGROUPS[g]
            n = len(chs)
            ch0 = chs[0]
            if g >= 2:
                v.wait_ge(s_pool, pool_pos[("cnt", g - 2)])  # t4 ring-2
            eb = eqb[g % 3]
            tb = t4[g % 2]
            in0 = AP(misc_t[:].tensor, ch0 * TOPK,
                     [[pstr(misc_t), 128], [TOPK, n], [0, E], [1, TOPK]])
            in1 = AP(misc_t[:].tensor, 128,
                     [[pstr(misc_t), 128], [0, n], [1, E * TOPK]])
            out_eq = AP(eb[:].tensor, 0,
                        [[pstr(eb), 128], [E * TOPK, n], [1, E * TOPK]])
            v.tensor_tensor(out_eq, in0, in1, op=mybir.AluOpType.is_equal)
            v.drain()
            a_in0 = AP(eb[:].tensor, 0,
                       [[pstr(eb), 128], [E * TOPK, n], [TOPK, E], [2, 4]])
            a_in1 = AP(eb[:].tensor, 1,
                       [[pstr(eb), 128], [E * TOPK, n], [TOPK, E], [2, 4]])
            a_out = AP(tb[:].tensor, 0,
                       [[pstr(tb), 128], [E * 4, n], [1, E * 4]])
            v.tensor_tensor(
                a_out, a_in0, a_in1, op=mybir.AluOpType.add
            ).then_inc(s_dve, 1)
            v.drain()
        # s_pe: per pair tr + per tile mm, group-blocked order
        pe_pos = {}
        pe_order = []
        p = 0
        for chs in EGROUPS:
            for pr in sorted({c // 2 for c in chs}):
                p += 1
                pe_pos[("tr", pr)] = p
                pe_order.append(("tr", pr))
            for c in chs:
                for q in range(4):
                    p += 1
                    pe_pos[("mm", 4 * c + q)] = p
                    pe_order.append(("mm", c, q))

        # copy plan: ACT dbls per bank except DVE chunks' h1 pair
        act_ops = []  # ("ctcp", pr) | ("dbl", c, h)
        ctcp_done = set()
        for c in range(NCH):
            pr = c // 2
            if pr not in ctcp_done:
                ctcp_done.add(pr)
                act_ops.append(("ctcp", pr))
            for h in range(2):
                if c in DVE_CHUNKS and h == 1:
                    continue  # DVE singles
                act_ops.append(("dbl", c, h))
        act_pos = {op: i + 1 for i, op in enumerate(act_ops)}

        tile_event = {}
        for op in act_ops:
            if op[0] == "dbl":
                _, c, h = op
                for t in (4 * c + 2 * h, 4 * c + 2 * h + 1):
                    tile_event[t] = ("scalar", act_pos[op])
        vcum = 0
        vpos = {}
        for t in DVE_TILES:
            vcum += 1
            vpos[t] = vcum
            tile_event[t] = ("vector", vcum)
        SEMS = {"scalar": s_cpa, "vector": s_cpv}

        # DVE copy weave: tiles of chunk c after mul C(pair(c+2)); both
        # q2/q3 singles. Safe: a group's trs precede its mms in PE order.
        weave = {}
        wtail = []
        for t in DVE_TILES:
            c_t = t // 4
            if t + NSLOT >= NT:
                wtail.append(t)
            else:
                weave.setdefault(min((c_t + 2) // 2, NPAIR - 1), []).append(t)

        def pct_tile(pr):
            return pct[pr % 2][:, (pr // 2) * 128 : (pr // 2) * 128 + 128]

        def pmm_slice(t, n=1):
            slot = t % NSLOT
            return pmm[slot // 2][:, (slot % 2) * 512 : (slot % 2) * 512 + 512 * n]

        def stg_slice(t, n=1):
            c, q = t // 4, t % 4
            return stg[c][:, q * 512 : (q + n) * 512]

        block = ctx.enter_context(nc.Block())

        @block.sync
        def _(sp):
            sp.dma_start(out=misc_t[:], in_=misc_d[:]).then_inc(s_ld[0], 16)
            sp.dma_start(out=sc_t[:], in_=sc_d[:]).then_inc(s_ld[1], 16)
            sp.dma_start(out=w_t[:, :1024], in_=wk_d[:, :1024]).then_inc(s_ld[2], 16)
            sp.dma_start(out=w_t[:, 1024:], in_=wk_d[:, 1024:]).then_inc(s_ld[3], 16)
            n_stores = 0
            for c in range(NCH):
                rows = slice(c * 128, (c + 1) * 128)
                pieces = [(0, 1024), (1024, 1024)] if c < 2 else [(0, COLS)]
                for col0, w in pieces:
                    needs: dict = {}
                    for t in range(col0 // 512, (col0 + w) // 512):
                        sem_name, thr = tile_event[4 * c + t]
                        needs[sem_name] = max(needs.get(sem_name, 0), thr)
                    for sem_name, thr in needs.items():
                        sp.wait_ge(SEMS[sem_name], thr)
                    sp.dma_start(
                        out=out_d[rows, col0 : col0 + w],
                        in_=stg[c][:, col0 : col0 + w],
                    ).then_inc(s_out, 16)
                    n_stores += 1
            sp.wait_ge(s_out, 16 * n_stores)

        @block.vector
        def _(v):
            def vcopy(t):
                v.wait_ge(s_pe, pe_pos[("mm", t)])
                v.tensor_copy(stg_slice(t), pmm_slice(t)).then_inc(s_cpv, 1)

            v.wait_ge(s_ld[0], 16)
            emit_eqa1(v, 0, mybir, AP)
            emit_eqa1(v, 1, mybir, AP)
            v.wait_ge(s_ld[1], 16)  # scores
            for g, chs in enumerate(EGROUPS):
                ch0 = chs[0]
                v.wait_ge(s_pool, pool_pos[("cnt", g)])
                cols = slice(ch0 * E, (chs[-1] + 1) * E)
                v.tensor_mul(cbuf[:, cols], cnt[:, cols], sc_t[:, cols])
                v.drain()
                for pr in sorted({c // 2 for c in chs}):
                    v.sem_inc(s_dve, 1)
                if g + 2 < len(EGROUPS):
                    emit_eqa1(v, g + 2, mybir, AP)
                for pr in sorted({c // 2 for c in chs}):
                    for t in weave.get(pr, ()):
                        vcopy(t)
            for t in wtail:
                vcopy(t)

        @block.gpsimd
        def _(gp):
            for g, chs in enumerate(EGROUPS):
                n = len(chs)
                ch0 = chs[0]
                gp.wait_ge(s_dve, dve_pos[("eqa1", g)])
                tb = t4[g % 2]
                b_in0 = AP(tb[:].tensor, 0,
                           [[pstr(tb), 128], [E * 4, n], [4, E], [2, 2]])
                b_in1 = AP(tb[:].tensor, 1,
                           [[pstr(tb), 128], [E * 4, n], [4, E], [2, 2]])
                b_out = AP(t2[:].tensor, 0,
                           [[pstr(t2), 128], [E * 2, n], [1, E * 2]])
                gp.tensor_tensor(b_out, b_in0, b_in1, op=mybir.AluOpType.add)
                gp.drain()
                c_in0 = AP(t2[:].tensor, 0,
                           [[pstr(t2), 128], [E * 2, n], [2, E]])
                c_in1 = AP(t2[:].tensor, 1,
                           [[pstr(t2), 128], [E * 2, n], [2, E]])
                c_out = AP(cnt[:].tensor, ch0 * E,
                           [[pstr(cnt), 128], [E, n], [1, E]])
                gp.tensor_tensor(
                    c_out, c_in0, c_in1, op=mybir.AluOpType.add
                ).then_inc(s_pool, 1)
                gp.drain()

        @block.tensor
        def _(t):
            for item in pe_order:
                if item[0] == "tr":
                    pr = item[1]
                    t.wait_ge(s_dve, dve_pos[("C", pr)])
                    if pr >= 2:  # pct ping-pong guard (see kernel notes)
                        t.wait_ge(s_cpa, act_pos[("ctcp", pr - 2)])
                    t.matmul(
                        pct_tile(pr),
                        cbuf[:, pr * 128 : (pr + 1) * 128],
                        ident,
                        is_transpose=True,
                        start=True,
                        stop=True,
                    ).then_inc(s_pe, 1)
                else:
                    _, c, q = item
                    tile = 4 * c + q
                    pr = c // 2
                    h = c % 2
                    if q == 0:
                        t.wait_ge(s_cpa, act_pos[("ctcp", pr)])
                    if tile < 2:
                        t.wait_ge(s_ld[2], 16)
                    if tile == 2:
                        t.wait_ge(s_ld[3], 16)
                    if tile >= NSLOT:
                        sem_name, thr = tile_event[tile - NSLOT]
                        t.wait_ge(SEMS[sem_name], thr)
                    t.matmul(
                        pmm_slice(tile),
                        ct[64 * h : 64 * h + 64, pr * 128 : (pr + 1) * 128],
                        w_t[64 * h : 64 * h + 64, q * 512 : (q + 1) * 512],
                        start=True,
                        stop=True,
                    ).then_inc(s_pe, 1)

        @block.scalar
        def _(a):
            for op in act_ops:
                if op[0] == "ctcp":
                    pr = op[1]
                    a.wait_ge(s_pe, pe_pos[("tr", pr)])
                    a.copy(
                        ct[:, pr * 128 : (pr + 1) * 128], pct_tile(pr)
                    ).then_inc(s_cpa, 1)
                else:
                    _, c, h = op
                    t0 = 4 * c + 2 * h
                    a.wait_ge(s_pe, pe_pos[("mm", t0 + 1)])
                    a.copy(stg_slice(t0, 2), pmm_slice(t0, 2)).then_inc(s_cpa, 1)

    return nc


def _host_prep(selection_score, expert_indices, all_weight):
    import ml_dtypes

    bf = ml_dtypes.bfloat16
    scores = np.asarray(selection_score, dtype=np.float32)
    idx = np.asarray(expert_indices).astype(np.int64)
    w = np.asarray(all_weight, dtype=np.float32).reshape(E, NF)

    idx_all = (
        idx.reshape(NCH, 128, TOPK).transpose(1, 0, 2).reshape(128, NCH * TOPK)
    ).astype(np.float32)
    iota = np.tile(np.repeat(np.arange(E, dtype=np.float32), TOPK), (128, 1))
    ident = np.eye(128, dtype=np.float32)
    misc = np.ascontiguousarray(
        np.concatenate([idx_all, iota, ident], axis=1).astype(bf)
    )
    sc = np.ascontiguousarray(
        scores.reshape(NCH, 128, E).transpose(1, 0, 2).reshape(128, NCH * E)
    ).astype(bf)
    wk_cores = []
    for core in range(N_CORES):
        sl = w[:, core * COLS : (core + 1) * COLS].astype(bf)
        wk_cores.append(np.ascontiguousarray(np.concatenate([sl, sl], axis=0)))
    return misc, sc, wk_cores


def _run(selection_score, expert_indices, all_weight, trace=False):
    from concourse.bass_utils import run_bass_kernel_spmd

    misc, sc, wk_cores = _host_prep(selection_score, expert_indices, all_weight)
    if "nc" not in _cache:
        _cache["nc"] = _build_program()
    nc = _cache["nc"]
    in_maps = [
        {"misc": misc, "sc": sc, "wk": wk_cores[c]} for c in range(N_CORES)
    ]
    r = run_bass_kernel_spmd(nc, in_maps, list(range(N_CORES)), trace=trace)
    full = np.concatenate(
        [np.asarray(r.results[c]["out"]) for c in range(N_CORES)], axis=1
    )
    return full.astype(np.float32).reshape(BS, PL, D), r


def kernel(selection_score, expert_indices, all_weight) -> np.ndarray:
    full, _ = _run(selection_score, expert_indices, all_weight, trace=False)
    return full
